# revision 20
# baseline (speedup 1.0000x reference)
"""Trainium2 Bass kernel for nn_Attention_Layer (dense transformer attention).

Computes, for X [N, D], Wq/Wk/Wv [D, D]:
    Q = X @ Wq.T ; K = X @ Wk.T ; V = X @ Wv.T
    O = softmax(Q @ K.T, axis=-1) @ V

Strategy (8 NeuronCores, SPMD single launch):
  - Shard rows of X across cores (N=8192 -> 1024 rows/core).
  - Each core computes K_b^T and V_b (each written to an internal DRAM
    bounce and all-gathered immediately, K first), then Q_b^T (kept in SBUF,
    its PE work hiding both collectives' latency).
  - Attention runs in the "transposed" layout: S^T[k, q] tiles are computed
    with K^T chunks stationary and Q^T moving; softmax uses a constant bias
    shift (exact after normalization; no per-row max needed since logits are
    bounded well inside fp32 exp range), so no on-chip transposes and no
    partition-axis reductions are ever needed.  P~ = exp(S^T + bias) chunks
    feed P@V directly as stationary operands; row-sums come from tiny N=2
    matmuls against a ones pair into a shared PSUM bank.  O accumulates in
    SBUF and is normalized once at the end.
  - All matmuls run as float32r (full PE rate at free-dim >= 256).

AllGather concatenates rank blocks on axis 0; keys are processed in rank-block
order on every core, and the same (rank, local-row) indexing is used for both
K^T and V, so the softmax/PV reduction is consistent (softmax is permutation
invariant over keys).
"""

import numpy as np

import concourse.tile as tile
from concourse import bacc, mybir
from concourse.bass_utils import run_bass_kernel_spmd

N_CORES = 8
N_TOTAL = 8192
D_MODEL = 1024
R_PER_CORE = N_TOTAL // N_CORES  # 1024

F32 = mybir.dt.float32
EXP_BIAS = -45.0  # constant softmax shift; cancels exactly after normalization


def _mm_dt(use_f32r):
    return mybir.dt.float32r if use_f32r else mybir.dt.float32


def build_fused(
    n_cores=N_CORES,
    d=D_MODEL,
    r=R_PER_CORE,
    kb=512,
    exp_bias=EXP_BIAS,
    use_f32r=True,
    mock_ag=False,    # timing/sim builds: skip the collective, read own kvb
    repeat_attn=1,    # timing builds: run stage B this many times
    stream_bufs=2,    # double/triple buffering of the streamed K^T/V tiles
    ps_a_bufs=8,      # stage-A psum pipelining depth
    pt_bufs=2,        # P~ tile double-buffering across key blocks
):
    """Build the fused QKV + AllGather + attention kernel (SPMD, one program).

    Per-core I/O:
      xt  [d, r]  ExternalInput  — X^T columns for this core's rows
      wqt/wkt/wvt [d, d] ExternalInput — W.T (replicated)
      o   [r, d]  ExternalOutput — this core's output rows
    """
    assert d % 128 == 0 and r % 128 == 0 and kb % 128 == 0
    DC = d // 128            # contraction chunks over d
    NQS = r // 128           # 128-query subtiles per core
    QG = min(512, r)         # query group (free dim) for S^T matmuls
    NQG = r // QG
    KC = kb // 128           # key chunks per key block
    BPR = r // kb            # key blocks per rank block
    DW = min(512, d)         # free-dim slice width over d
    ND = d // DW             # slices of d (for PV matmuls)
    RW = min(512, r)         # free-dim slice width over r
    NR = r // RW

    MM = _mm_dt(use_f32r)  # dtype of all matmul operands (producers round)

    nc = bacc.Bacc("TRN2", target_bir_lowering=False, debug=False, num_devices=n_cores)

    xt = nc.dram_tensor("xt", [d, r], MM, kind="ExternalInput").ap()
    wqt = nc.dram_tensor("wqt", [d, d], MM, kind="ExternalInput").ap()
    wkt = nc.dram_tensor("wkt", [d, d], MM, kind="ExternalInput").ap()
    wvt = nc.dram_tensor("wvt", [d, d], MM, kind="ExternalInput").ap()
    o = nc.dram_tensor("o", [r, d], F32, kind="ExternalOutput").ap()

    # Internal DRAM bounces: K_b^T and V_b, and their all-gathers.  Two
    # separate collectives so attention (which needs K^T + Q^T first) can
    # start while V is still gathering.
    ktb = nc.dram_tensor("ktb", [d, r], MM).ap()
    vb = nc.dram_tensor("vb", [r, d], MM).ap()
    ktg = nc.dram_tensor("ktg", [n_cores * d, r], MM, addr_space="Shared").ap()
    vg = nc.dram_tensor("vg", [n_cores * r, d], MM, addr_space="Shared").ap()

    with tile.TileContext(nc) as tc:
        with tc.tile_pool(name="persist", bufs=1) as pp:
            # --- persistent tiles ---
            qt_t = []
            for dc in range(DC):
                t = pp.tile([128, r], MM, name=f"qt{dc}", tag=f"qt{dc}")
                qt_t.append(t)
            oacc = []
            for qs in range(NQS):
                t = pp.tile([128, d], F32, name=f"oacc{qs}", tag=f"oacc{qs}")
                oacc.append(t)
            oacc_rs = pp.tile([128, 2 * NQS], F32, name="oacc_rs", tag="oacc_rs")
            # ones pair (fp32r matmuls need even free dims, so the row-sum
            # is computed twice into adjacent psum columns)
            ones_t = pp.tile([128, 2], MM, name="ones_t", tag="ones_t")
            bias_t = pp.tile([128, 1], F32, name="bias_t", tag="bias_t")
            nc.vector.memset(bias_t, exp_bias)
            ones_f32 = pp.tile([128, 2], F32, name="ones_f32", tag="ones_f32")
            nc.vector.memset(ones_f32, 1.0)
            nc.vector.tensor_copy(ones_t, ones_f32)

            # ---------------- Stage A: projections ----------------
            with (
                tc.tile_pool(name="stage_a", bufs=1) as pa,
                tc.tile_pool(name="ps_a", bufs=ps_a_bufs, space="PSUM") as ps_a,
                tc.tile_pool(name="outs_a", bufs=2) as pout_a,
            ):
                # All three weight sets stay resident (96KB/part) so every
                # load is issued up front and overlaps projection compute —
                # scoped/reused pools would serialize wv's DMAs behind the
                # last K-proj matmul.  Issue order: (wk, xt) pairs first so
                # the K projection can start as soon as chunk 0 lands.
                xt_t, w_t = [], {}
                for dc in range(DC):
                    t = pa.tile([128, d], MM, name=f"wk{dc}", tag=f"wk{dc}")
                    nc.sync.dma_start(out=t, in_=wkt[dc * 128:(dc + 1) * 128, :])
                    w_t[("k", dc)] = t
                    t = pa.tile([128, r], MM, name=f"xt{dc}", tag=f"xt{dc}")
                    nc.sync.dma_start(out=t, in_=xt[dc * 128:(dc + 1) * 128, :])
                    xt_t.append(t)
                for wname, wap in (("v", wvt), ("q", wqt)):
                    for dc in range(DC):
                        t = pa.tile([128, d], MM, name=f"w{wname}{dc}", tag=f"w{wname}{dc}")
                        nc.sync.dma_start(out=t, in_=wap[dc * 128:(dc + 1) * 128, :])
                        w_t[(wname, dc)] = t

                def proj_T(w_t, keep_tiles=None):
                    # out[do, r] = sum_di W^T[di, do] * X^T[di, r]  (i.e. (X@W.T)^T)
                    for oc in range(d // 128):
                        for rg in range(NR):
                            ps = ps_a.tile([128, RW], F32, name="ps", tag="ps")
                            for dc in range(DC):
                                nc.tensor.matmul(
                                    ps,
                                    w_t[dc][:, oc * 128:(oc + 1) * 128],
                                    xt_t[dc][:, rg * RW:(rg + 1) * RW],
                                    start=(dc == 0),
                                    stop=(dc == DC - 1),
                                )
                            if keep_tiles is not None:
                                nc.vector.tensor_copy(
                                    keep_tiles[oc][:, rg * RW:(rg + 1) * RW], ps
                                )
                            else:
                                ot = pout_a.tile([128, RW], MM, name="ot", tag="ot")
                                nc.vector.tensor_copy(ot, ps)
                                nc.sync.dma_start(
                                    out=ktb[oc * 128:(oc + 1) * 128,
                                            rg * RW:(rg + 1) * RW],
                                    in_=ot,
                                )

                # K_b^T -> ktb, then gather immediately
                proj_T({dc: w_t[("k", dc)] for dc in range(DC)})
                if not mock_ag:
                    nc.gpsimd.collective_compute(
                        "AllGather",
                        mybir.AluOpType.bypass,
                        ins=[ktb],
                        outs=[ktg],
                        replica_groups=[list(range(n_cores))],
                    )

                # V_b (natural layout) -> vb, then gather immediately so both
                # collectives are in flight while the Q projection runs; PV of
                # the first attention blocks then never waits on the gather.
                if True:
                    wv_t = {dc: w_t[("v", dc)] for dc in range(DC)}
                    for rc in range(r // 128):
                        for og in range(ND):
                            ps = ps_a.tile([128, DW], F32, name="ps", tag="ps")
                            for dc in range(DC):
                                nc.tensor.matmul(
                                    ps,
                                    xt_t[dc][:, rc * 128:(rc + 1) * 128],
                                    wv_t[dc][:, og * DW:(og + 1) * DW],
                                    start=(dc == 0),
                                    stop=(dc == DC - 1),
                                )
                            ot = pout_a.tile([128, DW], MM, name="ot", tag="ot2")
                            nc.vector.tensor_copy(ot, ps)
                            nc.sync.dma_start(
                                out=vb[rc * 128:(rc + 1) * 128,
                                       og * DW:(og + 1) * DW],
                                in_=ot,
                            )
                if not mock_ag:
                    nc.gpsimd.collective_compute(
                        "AllGather",
                        mybir.AluOpType.bypass,
                        ins=[vb],
                        outs=[vg],
                        replica_groups=[list(range(n_cores))],
                    )

                # Q_b^T stays in SBUF (PE work that overlaps both gathers)
                proj_T({dc: w_t[("q", dc)] for dc in range(DC)}, keep_tiles=qt_t)

            # ---------------- Stage B: attention ----------------
            with (
                tc.tile_pool(name="stream", bufs=stream_bufs) as pstream,
                tc.tile_pool(name="pt_pool", bufs=pt_bufs) as ppt,
                tc.tile_pool(name="ps_st", bufs=3, space="PSUM") as ps_st,
                tc.tile_pool(name="ps_pv", bufs=2, space="PSUM") as ps_pv,
                tc.tile_pool(name="ps_rs", bufs=1, space="PSUM") as ps_rs,
                tc.tile_pool(name="outp", bufs=2) as pout,
            ):
                n_blocks = n_cores * BPR
                for blk_i in range(repeat_attn * n_blocks):
                    blk = blk_i % n_blocks
                    rank = blk // BPR
                    half = blk % BPR
                    if mock_ag:
                        kt_src, v_src = ktb, vb
                        kt_row0 = 0
                        v_row0 = half * kb
                    else:
                        kt_src, v_src = ktg, vg
                        kt_row0 = rank * d            # K^T rows of this rank in ktg
                        v_row0 = rank * r + half * kb  # V rows for this block

                    kt_t = []
                    for dc in range(DC):
                        t = pstream.tile([128, kb], MM, name=f"kt{dc}", tag=f"kt{dc}")
                        nc.sync.dma_start(
                            out=t,
                            in_=kt_src[kt_row0 + dc * 128:kt_row0 + (dc + 1) * 128,
                                       half * kb:(half + 1) * kb],
                        )
                        kt_t.append(t)
                    v_t = []
                    for kc in range(KC):
                        t = pstream.tile([128, d], MM, name=f"v{kc}", tag=f"v{kc}")
                        nc.sync.dma_start(
                            out=t,
                            in_=v_src[v_row0 + kc * 128:v_row0 + (kc + 1) * 128, :],
                        )
                        v_t.append(t)

                    # S^T = K_chunk @ Q^T ; P~ = exp(S^T + bias)
                    pt_t = {}
                    for kc in range(KC):
                        for qg in range(NQG):
                            ps = ps_st.tile([128, QG], F32, name="st_ps", tag="st_ps")
                            for dc in range(DC):
                                nc.tensor.matmul(
                                    ps,
                                    kt_t[dc][:, kc * 128:(kc + 1) * 128],
                                    qt_t[dc][:, qg * QG:(qg + 1) * QG],
                                    start=(dc == 0),
                                    stop=(dc == DC - 1),
                                )
                            pt = ppt.tile([128, QG], MM, name="pt", tag=f"pt{kc}_{qg}")
                            nc.scalar.activation(
                                pt, ps, mybir.ActivationFunctionType.Exp,
                                bias=bias_t, scale=1.0,
                            )
                            pt_t[(kc, qg)] = pt

                    # O += P~^T.T @ V ; row-sums via ones into shared rs bank
                    rs = ps_rs.tile([128, 2 * NQS], F32, name="rs_ps", tag="rs_ps")
                    for qs in range(NQS):
                        qg, off = divmod(qs * 128, QG)
                        pv = ps_pv.tile([128, d], F32, name="pv_ps", tag="pv_ps")
                        for kc in range(KC):
                            lhsT = pt_t[(kc, qg)][:, off:off + 128]
                            for nd in range(ND):
                                nc.tensor.matmul(
                                    pv[:, nd * DW:(nd + 1) * DW],
                                    lhsT,
                                    v_t[kc][:, nd * DW:(nd + 1) * DW],
                                    start=(kc == 0),
                                    stop=(kc == KC - 1),
                                    skip_group_check=True,
                                )
                            nc.tensor.matmul(
                                rs[:, 2 * qs:2 * qs + 2],
                                lhsT,
                                ones_t,
                                start=(kc == 0),
                                stop=(kc == KC - 1),
                                skip_group_check=True,
                            )
                        if blk_i == 0:
                            nc.vector.tensor_copy(oacc[qs], pv)
                        else:
                            nc.vector.tensor_add(oacc[qs], oacc[qs], pv)
                    if blk_i == 0:
                        nc.vector.tensor_copy(oacc_rs, rs)
                    else:
                        nc.vector.tensor_add(oacc_rs, oacc_rs, rs)

                # normalize + write out
                recip = pout.tile([128, 2 * NQS], F32, name="recip", tag="recip", bufs=1)
                nc.vector.reciprocal(recip, oacc_rs)
                for qs in range(NQS):
                    ot = pout.tile([128, d], F32, name="ot", tag="ot")
                    nc.vector.tensor_scalar_mul(ot, oacc[qs], recip[:, 2 * qs:2 * qs + 1])
                    nc.sync.dma_start(out=o[qs * 128:(qs + 1) * 128, :], in_=ot)

    nc.compile()
    return nc


_NC_CACHE = {}


def _get_nc():
    if "fused" not in _NC_CACHE:
        _NC_CACHE["fused"] = build_fused()
    return _NC_CACHE["fused"]


def kernel(inputs, Wq, Wk, Wv):
    inputs = np.ascontiguousarray(inputs, dtype=np.float32)
    XT = np.ascontiguousarray(inputs.T)
    WqT = np.ascontiguousarray(np.asarray(Wq, dtype=np.float32).T)
    WkT = np.ascontiguousarray(np.asarray(Wk, dtype=np.float32).T)
    WvT = np.ascontiguousarray(np.asarray(Wv, dtype=np.float32).T)

    nc = _get_nc()
    R = R_PER_CORE
    in_maps = [
        {
            "xt": np.ascontiguousarray(XT[:, c * R:(c + 1) * R]),
            "wqt": WqT,
            "wkt": WkT,
            "wvt": WvT,
        }
        for c in range(N_CORES)
    ]
    res = run_bass_kernel_spmd(nc, in_maps, core_ids=list(range(N_CORES)))
    out = np.concatenate([res.results[c]["o"] for c in range(N_CORES)], axis=0)
    return out.astype(np.float32)


# revision 22
# speedup vs baseline: 1.0242x; 1.0242x over previous
"""Trainium2 Bass kernel for nn_Attention_Layer (dense transformer attention).

Computes, for X [N, D], Wq/Wk/Wv [D, D]:
    Q = X @ Wq.T ; K = X @ Wk.T ; V = X @ Wv.T
    O = softmax(Q @ K.T, axis=-1) @ V

Strategy (8 NeuronCores, SPMD single launch):
  - Shard rows of X across cores (N=8192 -> 1024 rows/core).
  - Each core computes K_b^T and V_b (each written to an internal DRAM
    bounce and all-gathered immediately, K first), then Q_b^T (kept in SBUF,
    its PE work hiding both collectives' latency).
  - Attention runs in the "transposed" layout: S^T[k, q] tiles are computed
    with K^T chunks stationary and Q^T moving; softmax uses a constant bias
    shift (exact after normalization; no per-row max needed since logits are
    bounded well inside fp32 exp range), so no on-chip transposes and no
    partition-axis reductions are ever needed.  P~ = exp(S^T + bias) chunks
    feed P@V directly as stationary operands; row-sums come from tiny N=2
    matmuls against a ones pair into a shared PSUM bank.  O accumulates in
    SBUF and is normalized once at the end.
  - All matmuls run as float32r (full PE rate at free-dim >= 256).

AllGather concatenates rank blocks on axis 0; keys are processed in rank-block
order on every core, and the same (rank, local-row) indexing is used for both
K^T and V, so the softmax/PV reduction is consistent (softmax is permutation
invariant over keys).
"""

import numpy as np

import concourse.tile as tile
from concourse import bacc, mybir
from concourse.bass_utils import run_bass_kernel_spmd

N_CORES = 8
N_TOTAL = 8192
D_MODEL = 1024
R_PER_CORE = N_TOTAL // N_CORES  # 1024

F32 = mybir.dt.float32
EXP_BIAS = -45.0  # constant softmax shift; cancels exactly after normalization


def _mm_dt(use_f32r):
    return mybir.dt.float32r if use_f32r else mybir.dt.float32


def build_fused(
    n_cores=N_CORES,
    d=D_MODEL,
    r=R_PER_CORE,
    kb=512,
    exp_bias=EXP_BIAS,
    use_f32r=True,
    mock_ag=False,    # timing/sim builds: skip the collective, read own kvb
    repeat_attn=1,    # timing builds: run stage B this many times
    stream_bufs=2,    # double/triple buffering of the streamed K^T/V tiles
    ps_a_bufs=8,      # stage-A psum pipelining depth
    pt_bufs=2,        # P~ tile double-buffering across key blocks
    split_dma=False,  # spread stage-A input loads across HWDGE+SWDGE queues
                      # (model-neutral; kept as an experiment knob)
):
    """Build the fused QKV + AllGather + attention kernel (SPMD, one program).

    Per-core I/O:
      xt  [d, r]  ExternalInput  — X^T columns for this core's rows
      wqt/wkt/wvt [d, d] ExternalInput — W.T (replicated)
      o   [r, d]  ExternalOutput — this core's output rows
    """
    assert d % 128 == 0 and r % 128 == 0 and kb % 128 == 0
    DC = d // 128            # contraction chunks over d
    NQS = r // 128           # 128-query subtiles per core
    QG = min(512, r)         # query group (free dim) for S^T matmuls
    NQG = r // QG
    KC = kb // 128           # key chunks per key block
    BPR = r // kb            # key blocks per rank block
    DW = min(512, d)         # free-dim slice width over d
    ND = d // DW             # slices of d (for PV matmuls)
    RW = min(512, r)         # free-dim slice width over r
    NR = r // RW

    MM = _mm_dt(use_f32r)  # dtype of all matmul operands (producers round)

    nc = bacc.Bacc("TRN2", target_bir_lowering=False, debug=False, num_devices=n_cores)

    xt = nc.dram_tensor("xt", [d, r], MM, kind="ExternalInput").ap()
    wqt = nc.dram_tensor("wqt", [d, d], MM, kind="ExternalInput").ap()
    wkt = nc.dram_tensor("wkt", [d, d], MM, kind="ExternalInput").ap()
    wvt = nc.dram_tensor("wvt", [d, d], MM, kind="ExternalInput").ap()
    o = nc.dram_tensor("o", [r, d], F32, kind="ExternalOutput").ap()

    # Internal DRAM bounces: K_b^T and V_b, and their all-gathers.  Two
    # separate collectives so attention (which needs K^T + Q^T first) can
    # start while V is still gathering.
    ktb = nc.dram_tensor("ktb", [d, r], MM).ap()
    vb = nc.dram_tensor("vb", [r, d], MM).ap()
    ktg = nc.dram_tensor("ktg", [n_cores * d, r], MM, addr_space="Shared").ap()
    vg = nc.dram_tensor("vg", [n_cores * r, d], MM, addr_space="Shared").ap()

    with tile.TileContext(nc) as tc:
        with tc.tile_pool(name="persist", bufs=1) as pp:
            # --- persistent tiles ---
            qt_t = []
            for dc in range(DC):
                t = pp.tile([128, r], MM, name=f"qt{dc}", tag=f"qt{dc}")
                qt_t.append(t)
            oacc = []
            for qs in range(NQS):
                t = pp.tile([128, d], F32, name=f"oacc{qs}", tag=f"oacc{qs}")
                oacc.append(t)
            oacc_rs = pp.tile([128, 2 * NQS], F32, name="oacc_rs", tag="oacc_rs")
            # ones pair (fp32r matmuls need even free dims, so the row-sum
            # is computed twice into adjacent psum columns)
            ones_t = pp.tile([128, 2], MM, name="ones_t", tag="ones_t")
            bias_t = pp.tile([128, 1], F32, name="bias_t", tag="bias_t")
            nc.vector.memset(bias_t, exp_bias)
            ones_f32 = pp.tile([128, 2], F32, name="ones_f32", tag="ones_f32")
            nc.vector.memset(ones_f32, 1.0)
            nc.vector.tensor_copy(ones_t, ones_f32)

            # ---------------- Stage A: projections ----------------
            with (
                tc.tile_pool(name="stage_a", bufs=1) as pa,
                tc.tile_pool(name="ps_a", bufs=ps_a_bufs, space="PSUM") as ps_a,
                tc.tile_pool(name="outs_a", bufs=2) as pout_a,
            ):
                # All three weight sets stay resident (96KB/part) so every
                # load is issued up front and overlaps projection compute —
                # scoped/reused pools would serialize wv's DMAs behind the
                # last K-proj matmul.  Issue order: (wk, xt) pairs first so
                # the K projection can start as soon as chunk 0 lands.
                xt_t, w_t = [], {}
                eng = [nc.sync, nc.gpsimd] if split_dma else [nc.sync, nc.sync]
                for dc in range(DC):
                    t = pa.tile([128, d], MM, name=f"wk{dc}", tag=f"wk{dc}")
                    eng[0].dma_start(out=t, in_=wkt[dc * 128:(dc + 1) * 128, :])
                    w_t[("k", dc)] = t
                    t = pa.tile([128, r], MM, name=f"xt{dc}", tag=f"xt{dc}")
                    eng[1].dma_start(out=t, in_=xt[dc * 128:(dc + 1) * 128, :])
                    xt_t.append(t)
                for wi, (wname, wap) in enumerate((("v", wvt), ("q", wqt))):
                    for dc in range(DC):
                        t = pa.tile([128, d], MM, name=f"w{wname}{dc}", tag=f"w{wname}{dc}")
                        eng[(dc + wi) % 2].dma_start(out=t, in_=wap[dc * 128:(dc + 1) * 128, :])
                        w_t[(wname, dc)] = t

                def proj_T(w_t, keep_tiles=None):
                    # out[do, r] = sum_di W^T[di, do] * X^T[di, r]  (i.e. (X@W.T)^T)
                    for oc in range(d // 128):
                        for rg in range(NR):
                            ps = ps_a.tile([128, RW], F32, name="ps", tag="ps")
                            for dc in range(DC):
                                nc.tensor.matmul(
                                    ps,
                                    w_t[dc][:, oc * 128:(oc + 1) * 128],
                                    xt_t[dc][:, rg * RW:(rg + 1) * RW],
                                    start=(dc == 0),
                                    stop=(dc == DC - 1),
                                )
                            if keep_tiles is not None:
                                nc.vector.tensor_copy(
                                    keep_tiles[oc][:, rg * RW:(rg + 1) * RW], ps
                                )
                            else:
                                ot = pout_a.tile([128, RW], MM, name="ot", tag="ot")
                                nc.vector.tensor_copy(ot, ps)
                                nc.sync.dma_start(
                                    out=ktb[oc * 128:(oc + 1) * 128,
                                            rg * RW:(rg + 1) * RW],
                                    in_=ot,
                                )

                # K_b^T -> ktb, then gather immediately
                proj_T({dc: w_t[("k", dc)] for dc in range(DC)})
                if not mock_ag:
                    nc.gpsimd.collective_compute(
                        "AllGather",
                        mybir.AluOpType.bypass,
                        ins=[ktb],
                        outs=[ktg],
                        replica_groups=[list(range(n_cores))],
                    )

                # V_b (natural layout) -> vb, then gather immediately so both
                # collectives are in flight while the Q projection runs; PV of
                # the first attention blocks then never waits on the gather.
                if True:
                    wv_t = {dc: w_t[("v", dc)] for dc in range(DC)}
                    for rc in range(r // 128):
                        for og in range(ND):
                            ps = ps_a.tile([128, DW], F32, name="ps", tag="ps")
                            for dc in range(DC):
                                nc.tensor.matmul(
                                    ps,
                                    xt_t[dc][:, rc * 128:(rc + 1) * 128],
                                    wv_t[dc][:, og * DW:(og + 1) * DW],
                                    start=(dc == 0),
                                    stop=(dc == DC - 1),
                                )
                            ot = pout_a.tile([128, DW], MM, name="ot", tag="ot2")
                            nc.vector.tensor_copy(ot, ps)
                            nc.sync.dma_start(
                                out=vb[rc * 128:(rc + 1) * 128,
                                       og * DW:(og + 1) * DW],
                                in_=ot,
                            )
                if not mock_ag:
                    nc.gpsimd.collective_compute(
                        "AllGather",
                        mybir.AluOpType.bypass,
                        ins=[vb],
                        outs=[vg],
                        replica_groups=[list(range(n_cores))],
                    )

                # Q_b^T stays in SBUF (PE work that overlaps both gathers)
                proj_T({dc: w_t[("q", dc)] for dc in range(DC)}, keep_tiles=qt_t)

            # ---------------- Stage B: attention ----------------
            with (
                tc.tile_pool(name="stream", bufs=stream_bufs) as pstream,
                tc.tile_pool(name="pt_pool", bufs=pt_bufs) as ppt,
                tc.tile_pool(name="ps_st", bufs=3, space="PSUM") as ps_st,
                tc.tile_pool(name="ps_pv", bufs=2, space="PSUM") as ps_pv,
                tc.tile_pool(name="ps_rs", bufs=1, space="PSUM") as ps_rs,
                tc.tile_pool(name="outp", bufs=2) as pout,
            ):
                n_blocks = n_cores * BPR
                for blk_i in range(repeat_attn * n_blocks):
                    blk = blk_i % n_blocks
                    rank = blk // BPR
                    half = blk % BPR
                    if mock_ag:
                        kt_src, v_src = ktb, vb
                        kt_row0 = 0
                        v_row0 = half * kb
                    else:
                        kt_src, v_src = ktg, vg
                        kt_row0 = rank * d            # K^T rows of this rank in ktg
                        v_row0 = rank * r + half * kb  # V rows for this block

                    kt_t = []
                    for dc in range(DC):
                        t = pstream.tile([128, kb], MM, name=f"kt{dc}", tag=f"kt{dc}")
                        nc.sync.dma_start(
                            out=t,
                            in_=kt_src[kt_row0 + dc * 128:kt_row0 + (dc + 1) * 128,
                                       half * kb:(half + 1) * kb],
                        )
                        kt_t.append(t)
                    v_t = []
                    for kc in range(KC):
                        t = pstream.tile([128, d], MM, name=f"v{kc}", tag=f"v{kc}")
                        nc.sync.dma_start(
                            out=t,
                            in_=v_src[v_row0 + kc * 128:v_row0 + (kc + 1) * 128, :],
                        )
                        v_t.append(t)

                    # S^T = K_chunk @ Q^T ; P~ = exp(S^T + bias)
                    pt_t = {}
                    for kc in range(KC):
                        for qg in range(NQG):
                            ps = ps_st.tile([128, QG], F32, name="st_ps", tag="st_ps")
                            for dc in range(DC):
                                nc.tensor.matmul(
                                    ps,
                                    kt_t[dc][:, kc * 128:(kc + 1) * 128],
                                    qt_t[dc][:, qg * QG:(qg + 1) * QG],
                                    start=(dc == 0),
                                    stop=(dc == DC - 1),
                                )
                            pt = ppt.tile([128, QG], MM, name="pt", tag=f"pt{kc}_{qg}")
                            nc.scalar.activation(
                                pt, ps, mybir.ActivationFunctionType.Exp,
                                bias=bias_t, scale=1.0,
                            )
                            pt_t[(kc, qg)] = pt

                    # O += P~^T.T @ V ; row-sums via ones into shared rs bank
                    rs = ps_rs.tile([128, 2 * NQS], F32, name="rs_ps", tag="rs_ps")
                    for qs in range(NQS):
                        qg, off = divmod(qs * 128, QG)
                        pv = ps_pv.tile([128, d], F32, name="pv_ps", tag="pv_ps")
                        for kc in range(KC):
                            lhsT = pt_t[(kc, qg)][:, off:off + 128]
                            for nd in range(ND):
                                nc.tensor.matmul(
                                    pv[:, nd * DW:(nd + 1) * DW],
                                    lhsT,
                                    v_t[kc][:, nd * DW:(nd + 1) * DW],
                                    start=(kc == 0),
                                    stop=(kc == KC - 1),
                                    skip_group_check=True,
                                )
                            nc.tensor.matmul(
                                rs[:, 2 * qs:2 * qs + 2],
                                lhsT,
                                ones_t,
                                start=(kc == 0),
                                stop=(kc == KC - 1),
                                skip_group_check=True,
                            )
                        if blk_i == 0:
                            nc.vector.tensor_copy(oacc[qs], pv)
                        else:
                            nc.vector.tensor_add(oacc[qs], oacc[qs], pv)
                    if blk_i == 0:
                        nc.vector.tensor_copy(oacc_rs, rs)
                    else:
                        nc.vector.tensor_add(oacc_rs, oacc_rs, rs)

                # normalize + write out
                recip = pout.tile([128, 2 * NQS], F32, name="recip", tag="recip", bufs=1)
                nc.vector.reciprocal(recip, oacc_rs)
                for qs in range(NQS):
                    ot = pout.tile([128, d], F32, name="ot", tag="ot")
                    nc.vector.tensor_scalar_mul(ot, oacc[qs], recip[:, 2 * qs:2 * qs + 1])
                    nc.sync.dma_start(out=o[qs * 128:(qs + 1) * 128, :], in_=ot)

    nc.compile()
    return nc


_NC_CACHE = {}


def _get_nc():
    if "fused" not in _NC_CACHE:
        _NC_CACHE["fused"] = build_fused()
    return _NC_CACHE["fused"]


def kernel(inputs, Wq, Wk, Wv):
    inputs = np.ascontiguousarray(inputs, dtype=np.float32)
    XT = np.ascontiguousarray(inputs.T)
    WqT = np.ascontiguousarray(np.asarray(Wq, dtype=np.float32).T)
    WkT = np.ascontiguousarray(np.asarray(Wk, dtype=np.float32).T)
    WvT = np.ascontiguousarray(np.asarray(Wv, dtype=np.float32).T)

    nc = _get_nc()
    R = R_PER_CORE
    in_maps = [
        {
            "xt": np.ascontiguousarray(XT[:, c * R:(c + 1) * R]),
            "wqt": WqT,
            "wkt": WkT,
            "wvt": WvT,
        }
        for c in range(N_CORES)
    ]
    res = run_bass_kernel_spmd(nc, in_maps, core_ids=list(range(N_CORES)))
    out = np.concatenate([res.results[c]["o"] for c in range(N_CORES)], axis=0)
    return out.astype(np.float32)


# revision 25
# speedup vs baseline: 1.0742x; 1.0488x over previous
"""Trainium2 Bass kernel for nn_Attention_Layer (dense transformer attention).

Computes, for X [N, D], Wq/Wk/Wv [D, D]:
    Q = X @ Wq.T ; K = X @ Wk.T ; V = X @ Wv.T
    O = softmax(Q @ K.T, axis=-1) @ V

Strategy (8 NeuronCores, SPMD single launch):
  - Shard rows of X across cores (N=8192 -> 1024 rows/core).
  - Each core computes K_b^T and V_b (each written to an internal DRAM
    bounce and all-gathered immediately, K first), then Q_b^T (kept in SBUF,
    its PE work hiding both collectives' latency).
  - Attention runs in the "transposed" layout: S^T[k, q] tiles are computed
    with K^T chunks stationary and Q^T moving; softmax uses a constant bias
    shift (exact after normalization; no per-row max needed since logits are
    bounded well inside fp32 exp range), so no on-chip transposes and no
    partition-axis reductions are ever needed.  P~ = exp(S^T + bias) chunks
    feed P@V directly as stationary operands; row-sums come from tiny N=2
    matmuls against a ones pair into a shared PSUM bank.  O accumulates in
    SBUF and is normalized once at the end.
  - All matmuls run as float32r (full PE rate at free-dim >= 256).

AllGather concatenates rank blocks on axis 0; keys are processed in rank-block
order on every core, and the same (rank, local-row) indexing is used for both
K^T and V, so the softmax/PV reduction is consistent (softmax is permutation
invariant over keys).
"""

import numpy as np

import concourse.tile as tile
from concourse import bacc, mybir
from concourse.bass_utils import run_bass_kernel_spmd

N_CORES = 8
N_TOTAL = 8192
D_MODEL = 1024
R_PER_CORE = N_TOTAL // N_CORES  # 1024

F32 = mybir.dt.float32
EXP_BIAS = -45.0  # constant softmax shift; cancels exactly after normalization


def _mm_dt(use_f32r):
    return mybir.dt.float32r if use_f32r else mybir.dt.float32


def build_fused(
    n_cores=N_CORES,
    d=D_MODEL,
    r=R_PER_CORE,
    kb=512,
    exp_bias=EXP_BIAS,
    use_f32r=True,
    mock_ag=False,    # timing/sim builds: skip the collective, read own kvb
    repeat_attn=1,    # timing builds: run stage B this many times
    stream_bufs=2,    # double/triple buffering of the streamed K^T/V tiles
    ps_a_bufs=8,      # stage-A psum pipelining depth
    pt_bufs=2,        # P~ tile double-buffering across key blocks
    split_dma=True,   # spread stage-A input loads across HWDGE+SWDGE queues
    tiny_loads=False, # COST-MODEL PROBE ONLY: stub out stage-A input DMAs
    halve_loads=True,   # issue stage-A loads as two half-width DMAs
):
    """Build the fused QKV + AllGather + attention kernel (SPMD, one program).

    Per-core I/O:
      xt  [d, r]  ExternalInput  — X^T columns for this core's rows
      wqt/wkt/wvt [d, d] ExternalInput — W.T (replicated)
      o   [r, d]  ExternalOutput — this core's output rows
    """
    assert d % 128 == 0 and r % 128 == 0 and kb % 128 == 0
    DC = d // 128            # contraction chunks over d
    NQS = r // 128           # 128-query subtiles per core
    QG = min(512, r)         # query group (free dim) for S^T matmuls
    NQG = r // QG
    KC = kb // 128           # key chunks per key block
    BPR = r // kb            # key blocks per rank block
    DW = min(512, d)         # free-dim slice width over d
    ND = d // DW             # slices of d (for PV matmuls)
    RW = min(512, r)         # free-dim slice width over r
    NR = r // RW

    MM = _mm_dt(use_f32r)  # dtype of all matmul operands (producers round)

    nc = bacc.Bacc("TRN2", target_bir_lowering=False, debug=False, num_devices=n_cores)

    xt = nc.dram_tensor("xt", [d, r], MM, kind="ExternalInput").ap()
    wqt = nc.dram_tensor("wqt", [d, d], MM, kind="ExternalInput").ap()
    wkt = nc.dram_tensor("wkt", [d, d], MM, kind="ExternalInput").ap()
    wvt = nc.dram_tensor("wvt", [d, d], MM, kind="ExternalInput").ap()
    o = nc.dram_tensor("o", [r, d], F32, kind="ExternalOutput").ap()

    # Internal DRAM bounces: K_b^T and V_b, and their all-gathers.  Two
    # separate collectives so attention (which needs K^T + Q^T first) can
    # start while V is still gathering.
    ktb = nc.dram_tensor("ktb", [d, r], MM).ap()
    vb = nc.dram_tensor("vb", [r, d], MM).ap()
    ktg = nc.dram_tensor("ktg", [n_cores * d, r], MM, addr_space="Shared").ap()
    vg = nc.dram_tensor("vg", [n_cores * r, d], MM, addr_space="Shared").ap()

    with tile.TileContext(nc) as tc:
        with tc.tile_pool(name="persist", bufs=1) as pp:
            # --- persistent tiles ---
            qt_t = []
            for dc in range(DC):
                t = pp.tile([128, r], MM, name=f"qt{dc}", tag=f"qt{dc}")
                qt_t.append(t)
            oacc = []
            for qs in range(NQS):
                t = pp.tile([128, d], F32, name=f"oacc{qs}", tag=f"oacc{qs}")
                oacc.append(t)
            oacc_rs = pp.tile([128, 2 * NQS], F32, name="oacc_rs", tag="oacc_rs")
            # ones pair (fp32r matmuls need even free dims, so the row-sum
            # is computed twice into adjacent psum columns)
            ones_t = pp.tile([128, 2], MM, name="ones_t", tag="ones_t")
            bias_t = pp.tile([128, 1], F32, name="bias_t", tag="bias_t")
            nc.vector.memset(bias_t, exp_bias)
            ones_f32 = pp.tile([128, 2], F32, name="ones_f32", tag="ones_f32")
            nc.vector.memset(ones_f32, 1.0)
            nc.vector.tensor_copy(ones_t, ones_f32)

            # ---------------- Stage A: projections ----------------
            with (
                tc.tile_pool(name="stage_a", bufs=1) as pa,
                tc.tile_pool(name="ps_a", bufs=ps_a_bufs, space="PSUM") as ps_a,
                tc.tile_pool(name="outs_a", bufs=2) as pout_a,
            ):
                # All three weight sets stay resident (96KB/part) so every
                # load is issued up front and overlaps projection compute —
                # scoped/reused pools would serialize wv's DMAs behind the
                # last K-proj matmul.  Issue order: (wk, xt) pairs first so
                # the K projection can start as soon as chunk 0 lands.
                xt_t, w_t = [], {}
                eng = [nc.sync, nc.gpsimd] if split_dma else [nc.sync, nc.sync]
                def load_in(e_idx, t, dram_rows, width):
                    if tiny_loads:
                        eng[e_idx % 2].dma_start(out=t[:, :8], in_=dram_rows[:, :8])
                    elif halve_loads:
                        h = width // 2
                        eng[e_idx % 2].dma_start(out=t[:, 0:h], in_=dram_rows[:, 0:h])
                        eng[(e_idx + 1) % 2].dma_start(out=t[:, h:width], in_=dram_rows[:, h:width])
                    else:
                        eng[e_idx % 2].dma_start(out=t, in_=dram_rows)

                for dc in range(DC):
                    t = pa.tile([128, d], MM, name=f"wk{dc}", tag=f"wk{dc}")
                    load_in(0, t, wkt[dc * 128:(dc + 1) * 128, :], d)
                    w_t[("k", dc)] = t
                    t = pa.tile([128, r], MM, name=f"xt{dc}", tag=f"xt{dc}")
                    load_in(1, t, xt[dc * 128:(dc + 1) * 128, :], r)
                    xt_t.append(t)
                for wi, (wname, wap) in enumerate((("v", wvt), ("q", wqt))):
                    for dc in range(DC):
                        t = pa.tile([128, d], MM, name=f"w{wname}{dc}", tag=f"w{wname}{dc}")
                        load_in(dc + wi, t, wap[dc * 128:(dc + 1) * 128, :], d)
                        w_t[(wname, dc)] = t

                def proj_T(w_t, keep_tiles=None):
                    # out[do, r] = sum_di W^T[di, do] * X^T[di, r]  (i.e. (X@W.T)^T)
                    for oc in range(d // 128):
                        for rg in range(NR):
                            ps = ps_a.tile([128, RW], F32, name="ps", tag="ps")
                            for dc in range(DC):
                                nc.tensor.matmul(
                                    ps,
                                    w_t[dc][:, oc * 128:(oc + 1) * 128],
                                    xt_t[dc][:, rg * RW:(rg + 1) * RW],
                                    start=(dc == 0),
                                    stop=(dc == DC - 1),
                                )
                            if keep_tiles is not None:
                                nc.vector.tensor_copy(
                                    keep_tiles[oc][:, rg * RW:(rg + 1) * RW], ps
                                )
                            else:
                                ot = pout_a.tile([128, RW], MM, name="ot", tag="ot")
                                nc.vector.tensor_copy(ot, ps)
                                nc.sync.dma_start(
                                    out=ktb[oc * 128:(oc + 1) * 128,
                                            rg * RW:(rg + 1) * RW],
                                    in_=ot,
                                )

                # K_b^T -> ktb, then gather immediately
                proj_T({dc: w_t[("k", dc)] for dc in range(DC)})
                if not mock_ag:
                    nc.gpsimd.collective_compute(
                        "AllGather",
                        mybir.AluOpType.bypass,
                        ins=[ktb],
                        outs=[ktg],
                        replica_groups=[list(range(n_cores))],
                    )

                # V_b (natural layout) -> vb, then gather immediately so both
                # collectives are in flight while the Q projection runs; PV of
                # the first attention blocks then never waits on the gather.
                if True:
                    wv_t = {dc: w_t[("v", dc)] for dc in range(DC)}
                    for rc in range(r // 128):
                        for og in range(ND):
                            ps = ps_a.tile([128, DW], F32, name="ps", tag="ps")
                            for dc in range(DC):
                                nc.tensor.matmul(
                                    ps,
                                    xt_t[dc][:, rc * 128:(rc + 1) * 128],
                                    wv_t[dc][:, og * DW:(og + 1) * DW],
                                    start=(dc == 0),
                                    stop=(dc == DC - 1),
                                )
                            ot = pout_a.tile([128, DW], MM, name="ot", tag="ot2")
                            nc.vector.tensor_copy(ot, ps)
                            nc.sync.dma_start(
                                out=vb[rc * 128:(rc + 1) * 128,
                                       og * DW:(og + 1) * DW],
                                in_=ot,
                            )
                if not mock_ag:
                    nc.gpsimd.collective_compute(
                        "AllGather",
                        mybir.AluOpType.bypass,
                        ins=[vb],
                        outs=[vg],
                        replica_groups=[list(range(n_cores))],
                    )

                # Q_b^T stays in SBUF (PE work that overlaps both gathers)
                proj_T({dc: w_t[("q", dc)] for dc in range(DC)}, keep_tiles=qt_t)

            # ---------------- Stage B: attention ----------------
            with (
                tc.tile_pool(name="stream", bufs=stream_bufs) as pstream,
                tc.tile_pool(name="pt_pool", bufs=pt_bufs) as ppt,
                tc.tile_pool(name="ps_st", bufs=3, space="PSUM") as ps_st,
                tc.tile_pool(name="ps_pv", bufs=2, space="PSUM") as ps_pv,
                tc.tile_pool(name="ps_rs", bufs=1, space="PSUM") as ps_rs,
                tc.tile_pool(name="outp", bufs=2) as pout,
            ):
                n_blocks = n_cores * BPR
                for blk_i in range(repeat_attn * n_blocks):
                    blk = blk_i % n_blocks
                    rank = blk // BPR
                    half = blk % BPR
                    if mock_ag:
                        kt_src, v_src = ktb, vb
                        kt_row0 = 0
                        v_row0 = half * kb
                    else:
                        kt_src, v_src = ktg, vg
                        kt_row0 = rank * d            # K^T rows of this rank in ktg
                        v_row0 = rank * r + half * kb  # V rows for this block

                    kt_t = []
                    for dc in range(DC):
                        t = pstream.tile([128, kb], MM, name=f"kt{dc}", tag=f"kt{dc}")
                        nc.sync.dma_start(
                            out=t,
                            in_=kt_src[kt_row0 + dc * 128:kt_row0 + (dc + 1) * 128,
                                       half * kb:(half + 1) * kb],
                        )
                        kt_t.append(t)
                    v_t = []
                    for kc in range(KC):
                        t = pstream.tile([128, d], MM, name=f"v{kc}", tag=f"v{kc}")
                        nc.sync.dma_start(
                            out=t,
                            in_=v_src[v_row0 + kc * 128:v_row0 + (kc + 1) * 128, :],
                        )
                        v_t.append(t)

                    # S^T = K_chunk @ Q^T ; P~ = exp(S^T + bias)
                    pt_t = {}
                    for kc in range(KC):
                        for qg in range(NQG):
                            ps = ps_st.tile([128, QG], F32, name="st_ps", tag="st_ps")
                            for dc in range(DC):
                                nc.tensor.matmul(
                                    ps,
                                    kt_t[dc][:, kc * 128:(kc + 1) * 128],
                                    qt_t[dc][:, qg * QG:(qg + 1) * QG],
                                    start=(dc == 0),
                                    stop=(dc == DC - 1),
                                )
                            pt = ppt.tile([128, QG], MM, name="pt", tag=f"pt{kc}_{qg}")
                            nc.scalar.activation(
                                pt, ps, mybir.ActivationFunctionType.Exp,
                                bias=bias_t, scale=1.0,
                            )
                            pt_t[(kc, qg)] = pt

                    # O += P~^T.T @ V ; row-sums via ones into shared rs bank
                    rs = ps_rs.tile([128, 2 * NQS], F32, name="rs_ps", tag="rs_ps")
                    for qs in range(NQS):
                        qg, off = divmod(qs * 128, QG)
                        pv = ps_pv.tile([128, d], F32, name="pv_ps", tag="pv_ps")
                        for kc in range(KC):
                            lhsT = pt_t[(kc, qg)][:, off:off + 128]
                            for nd in range(ND):
                                nc.tensor.matmul(
                                    pv[:, nd * DW:(nd + 1) * DW],
                                    lhsT,
                                    v_t[kc][:, nd * DW:(nd + 1) * DW],
                                    start=(kc == 0),
                                    stop=(kc == KC - 1),
                                    skip_group_check=True,
                                )
                            nc.tensor.matmul(
                                rs[:, 2 * qs:2 * qs + 2],
                                lhsT,
                                ones_t,
                                start=(kc == 0),
                                stop=(kc == KC - 1),
                                skip_group_check=True,
                            )
                        if blk_i == 0:
                            nc.vector.tensor_copy(oacc[qs], pv)
                        else:
                            nc.vector.tensor_add(oacc[qs], oacc[qs], pv)
                    if blk_i == 0:
                        nc.vector.tensor_copy(oacc_rs, rs)
                    else:
                        nc.vector.tensor_add(oacc_rs, oacc_rs, rs)

                # normalize + write out
                recip = pout.tile([128, 2 * NQS], F32, name="recip", tag="recip", bufs=1)
                nc.vector.reciprocal(recip, oacc_rs)
                for qs in range(NQS):
                    ot = pout.tile([128, d], F32, name="ot", tag="ot")
                    nc.vector.tensor_scalar_mul(ot, oacc[qs], recip[:, 2 * qs:2 * qs + 1])
                    nc.sync.dma_start(out=o[qs * 128:(qs + 1) * 128, :], in_=ot)

    nc.compile()
    return nc


_NC_CACHE = {}


def _get_nc():
    if "fused" not in _NC_CACHE:
        _NC_CACHE["fused"] = build_fused()
    return _NC_CACHE["fused"]


def kernel(inputs, Wq, Wk, Wv):
    inputs = np.ascontiguousarray(inputs, dtype=np.float32)
    XT = np.ascontiguousarray(inputs.T)
    WqT = np.ascontiguousarray(np.asarray(Wq, dtype=np.float32).T)
    WkT = np.ascontiguousarray(np.asarray(Wk, dtype=np.float32).T)
    WvT = np.ascontiguousarray(np.asarray(Wv, dtype=np.float32).T)

    nc = _get_nc()
    R = R_PER_CORE
    in_maps = [
        {
            "xt": np.ascontiguousarray(XT[:, c * R:(c + 1) * R]),
            "wqt": WqT,
            "wkt": WkT,
            "wvt": WvT,
        }
        for c in range(N_CORES)
    ]
    res = run_bass_kernel_spmd(nc, in_maps, core_ids=list(range(N_CORES)))
    out = np.concatenate([res.results[c]["o"] for c in range(N_CORES)], axis=0)
    return out.astype(np.float32)
